# revision 32
# baseline (speedup 1.0000x reference)
"""Multi-head attention + out-projection on 8 TRN2 NeuronCores.

Reference computation (per batch b, head h):
    S = Q K^T / sqrt(64);  P = softmax(S, axis=-1);  O = P V
    OUT = O @ W_out^T + b_out

Sharding: B*H = 64 (b,h) pairs split across 8 cores (8 pairs/core);
attention is fully local per pair, out-proj weights replicated.

Device-side structure (per core), measured ~329 us on silicon:
  - Pairs are processed two at a time (A,B) stacked in SBUF partitions
    0-63 / 64-127; the per-head S^T PSUM tiles act as each other's
    double buffer (PSUM is the binding constraint: 4 banks S + 4
    banks O/out-proj = all 8).
  - Every matmul is a full 128x128-array op so the PE activity monitor
    un-throttles to 2.4 GHz (half-array K=64/M=65 matmuls were stuck
    at the cold 1.2 GHz clock): K^T is zero-padded to 128 contraction
    rows (the zero rows annihilate the other head stacked in the qt
    rhs), and V is padded to 128 output columns.
  - S^T tiles [128 k, 1024 q] in PSUM; exp on ScalarE with the 1/8
    score scale folded into the activation; no max-subtraction
    (scores are O(+-7), exp stays inside f32/bf16 range). ScalarE is
    the bottleneck engine (~90% occupancy); everything else overlaps.
  - V carries a ones-column (col 64) so the PV matmul accumulates both
    O^T (partitions 0-63) and the softmax row-sums (partition 64).
  - Normalization: VectorE copies O^T+rowsum to SBUF (frees the PSUM
    accumulator early), a small DMA moves the rowsum row to partition
    0, gpsimd partition_broadcast replicates it, then VectorE
    reciprocal_approx_fast + multiply.
  - Out-proj once per pair: lhsT = normalized O^T slices, rhs =
    W_out^T -> natural [q, e] layout; bias added by VectorE in one
    [128, 1024] op. Its matmuls depend on the epilogue chain, so they
    are deferred and drip-fed into the next pair's first kt loop --
    otherwise the in-order PE stream stalls at every pair boundary
    and starves ScalarE. Input loads for the next pair are likewise
    issued ahead of the epilogue so the DMA queue prefetches first.

Host prep (plain numpy, free): Q/K pre-transposed to [d, s] bf16 with
K zero-padded per pair parity; V k-tiled p-major with ones-column and
zero padding, bf16; W_out^T bf16; bias pre-broadcast/tiled f32.
"""

import numpy as np
import ml_dtypes

from concourse import bacc, tile, mybir
from concourse.bass_utils import run_bass_kernel_spmd

B, H, S, D = 4, 16, 2048, 64
NCORES = 8
PAIRS = (B * H) // NCORES  # 8 (b,h) pairs per core
NKT = S // 128             # 16 key tiles
NQT = S // 128             # 16 query tiles
CHUNK = 1024               # query-column chunk (2 PSUM banks)
NCHUNK = S // CHUNK

_NC_CACHE = {}


def build_nc():
    f32, bf16 = mybir.dt.float32, mybir.dt.bfloat16
    nc = bacc.Bacc(None, target_bir_lowering=False)

    qt_d = nc.declare_dram_parameter("qt", [PAIRS, D, S], bf16, isOutput=False)
    kt_d = nc.declare_dram_parameter("kt", [PAIRS, 128, S], bf16, isOutput=False)
    vh_d = nc.declare_dram_parameter("vh", [PAIRS, 128, NKT, 128], bf16, isOutput=False)
    wt_d = nc.declare_dram_parameter("wt", [D, D], bf16, isOutput=False)
    bb_d = nc.declare_dram_parameter("bb", [128, NQT * D], f32, isOutput=False)
    out_d = nc.declare_dram_parameter("out", [PAIRS, 128, NQT * D], f32, isOutput=True)

    EXPF = mybir.ActivationFunctionType.Exp
    MULT = mybir.AluOpType.mult
    ADD = mybir.AluOpType.add

    with tile.TileContext(nc) as tc:
        with (
            tc.tile_pool(name="const", bufs=1) as constp,
            tc.tile_pool(name="qk", bufs=2) as qkp,
            tc.tile_pool(name="vhp", bufs=2) as vhp,
            tc.tile_pool(name="pt", bufs=12) as ptp,
            tc.tile_pool(name="ep", bufs=2) as epp,
            tc.tile_pool(name="osb", bufs=2) as osbp,
            tc.tile_pool(name="sApsum", bufs=1, space="PSUM") as sAp,
            tc.tile_pool(name="sBpsum", bufs=1, space="PSUM") as sBp,
            tc.tile_pool(name="opsum", bufs=2, space="PSUM") as opsum,
        ):
            wt_sb = constp.tile([D, D], bf16)
            bb_sb = constp.tile([128, NQT * D], f32)
            zb = constp.tile([128, 1], f32)
            nc.vector.memset(zb[:], 0.0)

            def load_pair(pq):
                pa, pb = 2 * pq, 2 * pq + 1
                qt2 = qkp.tile([128, S], bf16, tag="qt", name=f"qt_{pq}")
                kz2 = [
                    qkp.tile([128, S], bf16, tag="kza", name=f"kza_{pq}"),
                    qkp.tile([128, S], bf16, tag="kzb", name=f"kzb_{pq}"),
                ]
                vh2 = vhp.tile([128, 2, NKT, 128], bf16, name=f"vh_{pq}")
                nc.sync.dma_start(qt2[0:D, :], qt_d[pa])
                nc.sync.dma_start(qt2[D:128, :], qt_d[pb])
                nc.sync.dma_start(kz2[0][:], kt_d[pa])
                nc.sync.dma_start(kz2[1][:], kt_d[pb])
                nc.sync.dma_start(vh2[:, 0, :, :], vh_d[pa])
                nc.sync.dma_start(vh2[:, 1, :, :], vh_d[pb])
                return qt2, kz2, vh2

            loaded = load_pair(0)
            # consts are only needed by the (deferred) epilogues; load them
            # after the first pair's inputs so they don't head-of-line block.
            nc.sync.dma_start(wt_sb[:], wt_d[:])
            nc.sync.dma_start(bb_sb[:], bb_d[:])

            # Deferred out-projection of the previous pair: the opj matmuls
            # depend on the epilogue chain (copy->bcast->recip->mult), so
            # emitting them before the next pair's QK stream would stall the
            # PE (and starve ScalarE) at every pair boundary. Instead the
            # PSUM tiles are allocated at pair end (for slot cycling) and the
            # matmuls are drip-fed into the next pair's first kt loop.
            pending = None

            def emit_pending_step(step):
                opj2, on2p, out2p, pap, pbp = pending
                if step <= 8:
                    x, g = (0, step - 1) if step <= 4 else (1, step - 5)
                    for t in range(4 * g, 4 * g + 4):
                        nc.tensor.matmul(
                            opj2[x][:, t * D:(t + 1) * D],
                            on2p[x][:, t * 128:(t + 1) * 128],
                            wt_sb[:],
                            start=True, stop=True,
                        )
                elif step == 9:
                    nc.vector.tensor_tensor(out2p[0][:], opj2[0][:], bb_sb[:], ADD)
                    nc.sync.dma_start(out_d[pap], out2p[0][:])
                elif step == 10:
                    nc.vector.tensor_tensor(out2p[1][:], opj2[1][:], bb_sb[:], ADD)
                    nc.sync.dma_start(out_d[pbp], out2p[1][:])

            for pq in range(PAIRS // 2):
                pa, pb = 2 * pq, 2 * pq + 1
                qt2, kz2, vh2 = loaded
                out2 = [
                    osbp.tile([128, NQT * D], f32, tag="outA", name=f"out_{pq}_A"),
                    osbp.tile([128, NQT * D], f32, tag="outB", name=f"out_{pq}_B"),
                ]
                on2 = [
                    epp.tile([D, S], bf16, tag="onA", name=f"on_{pq}_A"),
                    epp.tile([D, S], bf16, tag="onB", name=f"on_{pq}_B"),
                ]

                for c in range(NCHUNK):
                    q0 = c * CHUNK
                    o_ps = [
                        opsum.tile([128, CHUNK], f32, tag="o", name=f"oA_{pq}_{c}"),
                        opsum.tile([128, CHUNK], f32, tag="o", name=f"oB_{pq}_{c}"),
                    ]
                    for k in range(NKT):
                        s_ps = [
                            sAp.tile([128, CHUNK], f32, tag="s", name=f"sA_{pq}_{c}_{k}"),
                            sBp.tile([128, CHUNK], f32, tag="s", name=f"sB_{pq}_{c}_{k}"),
                        ]
                        # j=0/j=1 share the stationary operand.
                        for x in (0, 1):
                            for j in (0, 1):
                                nc.tensor.matmul(
                                    s_ps[x][:, j * 512:(j + 1) * 512],
                                    kz2[x][:, k * 128:(k + 1) * 128],
                                    qt2[:, q0 + j * 512:q0 + (j + 1) * 512],
                                    start=True, stop=True,
                                )
                        p_sb = [None, None]
                        for x in (0, 1):
                            p_sb[x] = ptp.tile([128, CHUNK], bf16, tag="p", name=f"p_{pq}_{c}_{k}_{x}")
                            nc.scalar.activation(p_sb[x][:], s_ps[x][:], EXPF, bias=zb[:], scale=0.125)
                        for x in (0, 1):
                            for j in (0, 1):
                                nc.tensor.matmul(
                                    o_ps[x][:, j * 512:(j + 1) * 512],
                                    vh2[:, x, k, :],
                                    p_sb[x][:, j * 512:(j + 1) * 512],
                                    start=(k == 0), stop=(k == NKT - 1),
                                )
                        if c == 0 and pending is not None and 1 <= k <= 10:
                            emit_pending_step(k)
                            if k == 10:
                                pending = None

                    if c == NCHUNK - 1 and pq + 1 < PAIRS // 2:
                        loaded = load_pair(pq + 1)

                    last_tail = pq == PAIRS // 2 - 1 and c == NCHUNK - 1
                    for x in (0, 1):
                        o_sb = epp.tile([D + 1, CHUNK], f32, tag="osb", name=f"osb_{pq}_{c}_{x}")
                        if last_tail:
                            # ScalarE is idle after its final exp; use it for
                            # the copy so the tail's DVE chain shortens.
                            nc.scalar.copy(o_sb[:], o_ps[x][0:D + 1, :])
                        else:
                            nc.vector.tensor_copy(o_sb[:], o_ps[x][0:D + 1, :])
                        rs = epp.tile([1, CHUNK], f32, tag="rs", name=f"rs_{pq}_{c}_{x}")
                        nc.sync.dma_start(rs[:], o_sb[D:D + 1, :])
                        rb = epp.tile([D, CHUNK], f32, tag="rb", name=f"rb_{pq}_{c}_{x}")
                        nc.gpsimd.partition_broadcast(rb[:], rs[:])
                        nc.vector.reciprocal_approx_fast(rb[:], rb[:])
                        nc.vector.tensor_tensor(
                            on2[x][:, q0:q0 + CHUNK], o_sb[0:D, :], rb[:], MULT
                        )
                        if pq == PAIRS // 2 - 1 and c == NCHUNK - 1:
                            opj = opsum.tile(
                                [128, NQT * D], f32, tag="o", name=f"opjL_{x}"
                            )
                            for t in range(NQT):
                                nc.tensor.matmul(
                                    opj[:, t * D:(t + 1) * D],
                                    on2[x][:, t * 128:(t + 1) * 128],
                                    wt_sb[:],
                                    start=True, stop=True,
                                )
                            nc.vector.tensor_tensor(out2[x][:], opj[:], bb_sb[:], ADD)
                            nc.sync.dma_start(out_d[[pa, pb][x]], out2[x][:])

                if pq < PAIRS // 2 - 1:
                    opj2 = [
                        opsum.tile([128, NQT * D], f32, tag="o", name=f"opj_{pq}_A"),
                        opsum.tile([128, NQT * D], f32, tag="o", name=f"opj_{pq}_B"),
                    ]
                    pending = (opj2, on2, out2, pa, pb)

    nc.compile()
    return nc


def kernel(queries, keys, values, W_out, b_out):
    bf16 = ml_dtypes.bfloat16

    q = np.asarray(queries, dtype=np.float32).reshape(B * H, S, D)
    k = np.asarray(keys, dtype=np.float32).reshape(B * H, S, D)
    v = np.asarray(values, dtype=np.float32).reshape(B * H, S, D)

    wt = np.ascontiguousarray(np.asarray(W_out, dtype=np.float32).T).astype(bf16)
    bb = np.ascontiguousarray(
        np.tile(np.asarray(b_out, dtype=np.float32), (128, NQT))
    )

    in_maps = []
    for c in range(NCORES):
        sl = slice(c * PAIRS, (c + 1) * PAIRS)
        qt = np.ascontiguousarray(q[sl].transpose(0, 2, 1)).astype(bf16)
        # K^T zero-padded to 128 contraction rows: even pairs occupy rows
        # 0-63, odd pairs rows 64-127 (matching their slot in the stacked
        # qt2 rhs; the zero rows annihilate the other head's queries).
        kt = np.zeros((PAIRS, 128, S), dtype=bf16)
        for pp in range(PAIRS):
            r0 = (pp % 2) * D
            kt[pp, r0:r0 + D] = k[sl][pp].T.astype(bf16)
        # [pairs, S, D] -> k-tiled p-major [pairs, 128, NKT, 128]: cols 0-63
        # V, col 64 ones (softmax denominator), cols 65-127 zero padding.
        vt = v[sl].reshape(PAIRS, NKT, 128, D).transpose(0, 2, 1, 3)
        vh = np.zeros((PAIRS, 128, NKT, 128), dtype=bf16)
        vh[..., :D] = vt.astype(bf16)
        vh[..., D] = 1.0
        in_maps.append({"qt": qt, "kt": kt, "vh": vh, "wt": wt, "bb": bb})

    if "nc" not in _NC_CACHE:
        _NC_CACHE["nc"] = build_nc()
    nc = _NC_CACHE["nc"]

    global _LAST_IN_MAPS
    _LAST_IN_MAPS = in_maps

    res = run_bass_kernel_spmd(nc, in_maps, list(range(NCORES)))

    out = np.empty((B * H, S, D), dtype=np.float32)
    for c in range(NCORES):
        o = res.results[c]["out"]  # [PAIRS, 128, NQT*D], q = t*128 + p
        out[c * PAIRS:(c + 1) * PAIRS] = (
            o.reshape(PAIRS, 128, NQT, D).transpose(0, 2, 1, 3).reshape(PAIRS, S, D)
        )
    return out.reshape(B, H, S, D)


# revision 33
# speedup vs baseline: 1.0111x; 1.0111x over previous
"""Multi-head attention + out-projection on 8 TRN2 NeuronCores.

Reference computation (per batch b, head h):
    S = Q K^T / sqrt(64);  P = softmax(S, axis=-1);  O = P V
    OUT = O @ W_out^T + b_out

Sharding: B*H = 64 (b,h) pairs split across 8 cores (8 pairs/core);
attention is fully local per pair, out-proj weights replicated.

Device-side structure (per core), measured ~329 us on silicon:
  - Pairs are processed two at a time (A,B) stacked in SBUF partitions
    0-63 / 64-127; the per-head S^T PSUM tiles act as each other's
    double buffer (PSUM is the binding constraint: 4 banks S + 4
    banks O/out-proj = all 8).
  - Every matmul is a full 128x128-array op so the PE activity monitor
    un-throttles to 2.4 GHz (half-array K=64/M=65 matmuls were stuck
    at the cold 1.2 GHz clock): K^T is zero-padded to 128 contraction
    rows (the zero rows annihilate the other head stacked in the qt
    rhs), and V is padded to 128 output columns.
  - S^T tiles [128 k, 1024 q] in PSUM; exp on ScalarE with the 1/8
    score scale folded into the activation; no max-subtraction
    (scores are O(+-7), exp stays inside f32/bf16 range). ScalarE is
    the bottleneck engine (~90% occupancy); everything else overlaps.
  - V carries a ones-column (col 64) so the PV matmul accumulates both
    O^T (partitions 0-63) and the softmax row-sums (partition 64).
  - Normalization: VectorE copies O^T+rowsum to SBUF (frees the PSUM
    accumulator early), a small DMA moves the rowsum row to partition
    0, gpsimd partition_broadcast replicates it, then VectorE
    reciprocal_approx_fast + multiply.
  - Out-proj once per pair: lhsT = normalized O^T slices, rhs =
    W_out^T -> natural [q, e] layout; bias added by VectorE in one
    [128, 1024] op. Its matmuls depend on the epilogue chain, so they
    are deferred and drip-fed into the next pair's first kt loop --
    otherwise the in-order PE stream stalls at every pair boundary
    and starves ScalarE. Input loads for the next pair are likewise
    issued ahead of the epilogue so the DMA queue prefetches first.

Host prep (plain numpy, free): Q/K pre-transposed to [d, s] bf16 with
K zero-padded per pair parity; V k-tiled p-major with ones-column and
zero padding, bf16; W_out^T bf16; bias pre-broadcast/tiled f32.
"""

import numpy as np
import ml_dtypes

from concourse import bacc, tile, mybir
from concourse.bass_utils import run_bass_kernel_spmd

B, H, S, D = 4, 16, 2048, 64
NCORES = 8
PAIRS = (B * H) // NCORES  # 8 (b,h) pairs per core
NKT = S // 128             # 16 key tiles
NQT = S // 128             # 16 query tiles
CHUNK = 1024               # query-column chunk (2 PSUM banks)
NCHUNK = S // CHUNK

_NC_CACHE = {}


def build_nc():
    f32, bf16 = mybir.dt.float32, mybir.dt.bfloat16
    nc = bacc.Bacc(None, target_bir_lowering=False)

    qt_d = nc.declare_dram_parameter("qt", [PAIRS, D, S], bf16, isOutput=False)
    kt_d = nc.declare_dram_parameter("kt", [PAIRS, 128, S], bf16, isOutput=False)
    vh_d = nc.declare_dram_parameter("vh", [PAIRS, 128, NKT, 128], bf16, isOutput=False)
    wt_d = nc.declare_dram_parameter("wt", [D, D], bf16, isOutput=False)
    bb_d = nc.declare_dram_parameter("bb", [128, NQT * D], f32, isOutput=False)
    out_d = nc.declare_dram_parameter("out", [PAIRS, 128, NQT * D], f32, isOutput=True)

    EXPF = mybir.ActivationFunctionType.Exp
    MULT = mybir.AluOpType.mult
    ADD = mybir.AluOpType.add

    with tile.TileContext(nc) as tc:
        with (
            tc.tile_pool(name="const", bufs=1) as constp,
            tc.tile_pool(name="qk", bufs=2) as qkp,
            tc.tile_pool(name="vhp", bufs=2) as vhp,
            tc.tile_pool(name="pt", bufs=12) as ptp,
            tc.tile_pool(name="ep", bufs=2) as epp,
            tc.tile_pool(name="osb", bufs=2) as osbp,
            tc.tile_pool(name="sApsum", bufs=1, space="PSUM") as sAp,
            tc.tile_pool(name="sBpsum", bufs=1, space="PSUM") as sBp,
            tc.tile_pool(name="opsum", bufs=2, space="PSUM") as opsum,
        ):
            wt_sb = constp.tile([D, D], bf16)
            bb_sb = constp.tile([128, NQT * D], f32)
            zb = constp.tile([128, 1], f32)
            nc.vector.memset(zb[:], 0.0)

            def load_pair(pq):
                pa, pb = 2 * pq, 2 * pq + 1
                qt2 = qkp.tile([128, S], bf16, tag="qt", name=f"qt_{pq}")
                kz2 = [
                    qkp.tile([128, S], bf16, tag="kza", name=f"kza_{pq}"),
                    qkp.tile([128, S], bf16, tag="kzb", name=f"kzb_{pq}"),
                ]
                vh2 = vhp.tile([128, 2, NKT, 128], bf16, name=f"vh_{pq}")
                nc.sync.dma_start(qt2[0:D, :], qt_d[pa])
                nc.sync.dma_start(qt2[D:128, :], qt_d[pb])
                nc.sync.dma_start(kz2[0][:], kt_d[pa])
                nc.sync.dma_start(kz2[1][:], kt_d[pb])
                nc.sync.dma_start(vh2[:, 0, :, :], vh_d[pa])
                nc.sync.dma_start(vh2[:, 1, :, :], vh_d[pb])
                return qt2, kz2, vh2

            loaded = load_pair(0)
            # consts are only needed by the (deferred) epilogues; load them
            # after the first pair's inputs so they don't head-of-line block.
            nc.sync.dma_start(wt_sb[:], wt_d[:])
            nc.sync.dma_start(bb_sb[:], bb_d[:])

            # Deferred out-projection of the previous pair: the opj matmuls
            # depend on the epilogue chain (copy->bcast->recip->mult), so
            # emitting them before the next pair's QK stream would stall the
            # PE (and starve ScalarE) at every pair boundary. Instead the
            # PSUM tiles are allocated at pair end (for slot cycling) and the
            # matmuls are drip-fed into the next pair's first kt loop.
            pending = None

            def emit_pending_step(step):
                opj2, on2p, out2p, pap, pbp = pending
                if step <= 8:
                    x, g = (0, step - 1) if step <= 4 else (1, step - 5)
                    for t in range(4 * g, 4 * g + 4):
                        nc.tensor.matmul(
                            opj2[x][:, t * D:(t + 1) * D],
                            on2p[x][:, t * 128:(t + 1) * 128],
                            wt_sb[:],
                            start=True, stop=True,
                        )
                elif step == 9:
                    nc.vector.tensor_tensor(out2p[0][:], opj2[0][:], bb_sb[:], ADD)
                    nc.sync.dma_start(out_d[pap], out2p[0][:])
                elif step == 10:
                    nc.vector.tensor_tensor(out2p[1][:], opj2[1][:], bb_sb[:], ADD)
                    nc.sync.dma_start(out_d[pbp], out2p[1][:])

            for pq in range(PAIRS // 2):
                pa, pb = 2 * pq, 2 * pq + 1
                qt2, kz2, vh2 = loaded
                out2 = [
                    osbp.tile([128, NQT * D], f32, tag="outA", name=f"out_{pq}_A"),
                    osbp.tile([128, NQT * D], f32, tag="outB", name=f"out_{pq}_B"),
                ]
                on2 = [
                    epp.tile([D, S], bf16, tag="onA", name=f"on_{pq}_A"),
                    epp.tile([D, S], bf16, tag="onB", name=f"on_{pq}_B"),
                ]

                for c in range(NCHUNK):
                    q0 = c * CHUNK
                    o_ps = [
                        opsum.tile([128, CHUNK], f32, tag="o", name=f"oA_{pq}_{c}"),
                        opsum.tile([128, CHUNK], f32, tag="o", name=f"oB_{pq}_{c}"),
                    ]
                    for k in range(NKT):
                        s_ps = [
                            sAp.tile([128, CHUNK], f32, tag="s", name=f"sA_{pq}_{c}_{k}"),
                            sBp.tile([128, CHUNK], f32, tag="s", name=f"sB_{pq}_{c}_{k}"),
                        ]
                        # j=0/j=1 share the stationary operand.
                        for x in (0, 1):
                            for j in (0, 1):
                                nc.tensor.matmul(
                                    s_ps[x][:, j * 512:(j + 1) * 512],
                                    kz2[x][:, k * 128:(k + 1) * 128],
                                    qt2[:, q0 + j * 512:q0 + (j + 1) * 512],
                                    start=True, stop=True,
                                )
                        p_sb = [None, None]
                        for x in (0, 1):
                            p_sb[x] = ptp.tile([128, CHUNK], bf16, tag="p", name=f"p_{pq}_{c}_{k}_{x}")
                            nc.scalar.activation(p_sb[x][:], s_ps[x][:], EXPF, bias=zb[:], scale=0.125)
                        for x in (0, 1):
                            for j in (0, 1):
                                nc.tensor.matmul(
                                    o_ps[x][:, j * 512:(j + 1) * 512],
                                    vh2[:, x, k, :],
                                    p_sb[x][:, j * 512:(j + 1) * 512],
                                    start=(k == 0), stop=(k == NKT - 1),
                                )
                        if c == 0 and pending is not None and 1 <= k <= 10:
                            emit_pending_step(k)
                            if k == 10:
                                pending = None

                    if c == NCHUNK - 1 and pq + 1 < PAIRS // 2:
                        loaded = load_pair(pq + 1)

                    for x in (0, 1):
                        o_sb = epp.tile([D + 1, CHUNK], f32, tag="osb", name=f"osb_{pq}_{c}_{x}")
                        nc.vector.tensor_copy(o_sb[:], o_ps[x][0:D + 1, :])
                        rs = epp.tile([1, CHUNK], f32, tag="rs", name=f"rs_{pq}_{c}_{x}")
                        nc.sync.dma_start(rs[:], o_sb[D:D + 1, :])
                        rb = epp.tile([D, CHUNK], f32, tag="rb", name=f"rb_{pq}_{c}_{x}")
                        nc.gpsimd.partition_broadcast(rb[:], rs[:])
                        nc.vector.reciprocal_approx_fast(rb[:], rb[:])
                        nc.vector.tensor_tensor(
                            on2[x][:, q0:q0 + CHUNK], o_sb[0:D, :], rb[:], MULT
                        )
                        if pq == PAIRS // 2 - 1 and c == NCHUNK - 1:
                            opj = opsum.tile(
                                [128, NQT * D], f32, tag="o", name=f"opjL_{x}"
                            )
                            for t in range(NQT):
                                nc.tensor.matmul(
                                    opj[:, t * D:(t + 1) * D],
                                    on2[x][:, t * 128:(t + 1) * 128],
                                    wt_sb[:],
                                    start=True, stop=True,
                                )
                            nc.vector.tensor_tensor(out2[x][:], opj[:], bb_sb[:], ADD)
                            nc.sync.dma_start(out_d[[pa, pb][x]], out2[x][:])

                if pq < PAIRS // 2 - 1:
                    opj2 = [
                        opsum.tile([128, NQT * D], f32, tag="o", name=f"opj_{pq}_A"),
                        opsum.tile([128, NQT * D], f32, tag="o", name=f"opj_{pq}_B"),
                    ]
                    pending = (opj2, on2, out2, pa, pb)

    nc.compile()
    return nc


def kernel(queries, keys, values, W_out, b_out):
    bf16 = ml_dtypes.bfloat16

    q = np.asarray(queries, dtype=np.float32).reshape(B * H, S, D)
    k = np.asarray(keys, dtype=np.float32).reshape(B * H, S, D)
    v = np.asarray(values, dtype=np.float32).reshape(B * H, S, D)

    wt = np.ascontiguousarray(np.asarray(W_out, dtype=np.float32).T).astype(bf16)
    bb = np.ascontiguousarray(
        np.tile(np.asarray(b_out, dtype=np.float32), (128, NQT))
    )

    in_maps = []
    for c in range(NCORES):
        sl = slice(c * PAIRS, (c + 1) * PAIRS)
        qt = np.ascontiguousarray(q[sl].transpose(0, 2, 1)).astype(bf16)
        # K^T zero-padded to 128 contraction rows: even pairs occupy rows
        # 0-63, odd pairs rows 64-127 (matching their slot in the stacked
        # qt2 rhs; the zero rows annihilate the other head's queries).
        kt = np.zeros((PAIRS, 128, S), dtype=bf16)
        for pp in range(PAIRS):
            r0 = (pp % 2) * D
            kt[pp, r0:r0 + D] = k[sl][pp].T.astype(bf16)
        # [pairs, S, D] -> k-tiled p-major [pairs, 128, NKT, 128]: cols 0-63
        # V, col 64 ones (softmax denominator), cols 65-127 zero padding.
        vt = v[sl].reshape(PAIRS, NKT, 128, D).transpose(0, 2, 1, 3)
        vh = np.zeros((PAIRS, 128, NKT, 128), dtype=bf16)
        vh[..., :D] = vt.astype(bf16)
        vh[..., D] = 1.0
        in_maps.append({"qt": qt, "kt": kt, "vh": vh, "wt": wt, "bb": bb})

    if "nc" not in _NC_CACHE:
        _NC_CACHE["nc"] = build_nc()
    nc = _NC_CACHE["nc"]

    global _LAST_IN_MAPS
    _LAST_IN_MAPS = in_maps

    res = run_bass_kernel_spmd(nc, in_maps, list(range(NCORES)))

    out = np.empty((B * H, S, D), dtype=np.float32)
    for c in range(NCORES):
        o = res.results[c]["out"]  # [PAIRS, 128, NQT*D], q = t*128 + p
        out[c * PAIRS:(c + 1) * PAIRS] = (
            o.reshape(PAIRS, 128, NQT, D).transpose(0, 2, 1, 3).reshape(PAIRS, S, D)
        )
    return out.reshape(B, H, S, D)


# revision 34
# speedup vs baseline: 1.0112x; 1.0001x over previous
"""Multi-head attention + out-projection on 8 TRN2 NeuronCores.

Reference computation (per batch b, head h):
    S = Q K^T / sqrt(64);  P = softmax(S, axis=-1);  O = P V
    OUT = O @ W_out^T + b_out

Sharding: B*H = 64 (b,h) pairs split across 8 cores (8 pairs/core);
attention is fully local per pair, out-proj weights replicated.

Device-side structure (per core), measured ~329 us on silicon:
  - Pairs are processed two at a time (A,B) stacked in SBUF partitions
    0-63 / 64-127; the per-head S^T PSUM tiles act as each other's
    double buffer (PSUM is the binding constraint: 4 banks S + 4
    banks O/out-proj = all 8).
  - Every matmul is a full 128x128-array op so the PE activity monitor
    un-throttles to 2.4 GHz (half-array K=64/M=65 matmuls were stuck
    at the cold 1.2 GHz clock): K^T is zero-padded to 128 contraction
    rows (the zero rows annihilate the other head stacked in the qt
    rhs), and V is padded to 128 output columns.
  - S^T tiles [128 k, 1024 q] in PSUM; exp on ScalarE with the 1/8
    score scale folded into the activation; no max-subtraction
    (scores are O(+-7), exp stays inside f32/bf16 range). ScalarE is
    the bottleneck engine (~90% occupancy); everything else overlaps.
  - V carries a ones-column (col 64) so the PV matmul accumulates both
    O^T (partitions 0-63) and the softmax row-sums (partition 64).
  - Normalization: VectorE copies O^T+rowsum to SBUF (frees the PSUM
    accumulator early), a small DMA moves the rowsum row to partition
    0, gpsimd partition_broadcast replicates it, then VectorE
    reciprocal_approx_fast + multiply.
  - Out-proj once per pair: lhsT = normalized O^T slices, rhs =
    W_out^T -> natural [q, e] layout; bias added by VectorE in one
    [128, 1024] op. Its matmuls depend on the epilogue chain, so they
    are deferred and drip-fed into the next pair's first kt loop --
    otherwise the in-order PE stream stalls at every pair boundary
    and starves ScalarE. Input loads for the next pair are likewise
    issued ahead of the epilogue so the DMA queue prefetches first.

Host prep (plain numpy, free): Q/K pre-transposed to [d, s] bf16 with
K zero-padded per pair parity; V k-tiled p-major with ones-column and
zero padding, bf16; W_out^T bf16; bias pre-broadcast/tiled f32.
"""

import numpy as np
import ml_dtypes

from concourse import bacc, tile, mybir
from concourse.bass_utils import run_bass_kernel_spmd

B, H, S, D = 4, 16, 2048, 64
NCORES = 8
PAIRS = (B * H) // NCORES  # 8 (b,h) pairs per core
NKT = S // 128             # 16 key tiles
NQT = S // 128             # 16 query tiles
CHUNK = 1024               # query-column chunk (2 PSUM banks)
NCHUNK = S // CHUNK

_NC_CACHE = {}


def build_nc():
    f32, bf16 = mybir.dt.float32, mybir.dt.bfloat16
    nc = bacc.Bacc(None, target_bir_lowering=False)

    qt_d = nc.declare_dram_parameter("qt", [PAIRS, D, S], bf16, isOutput=False)
    kt_d = nc.declare_dram_parameter("kt", [PAIRS, 128, S], bf16, isOutput=False)
    vh_d = nc.declare_dram_parameter("vh", [PAIRS, 128, NKT, 128], bf16, isOutput=False)
    wt_d = nc.declare_dram_parameter("wt", [D, D], bf16, isOutput=False)
    bb_d = nc.declare_dram_parameter("bb", [128, NQT * D], f32, isOutput=False)
    out_d = nc.declare_dram_parameter("out", [PAIRS, 128, NQT * D], f32, isOutput=True)

    EXPF = mybir.ActivationFunctionType.Exp
    MULT = mybir.AluOpType.mult
    ADD = mybir.AluOpType.add

    with tile.TileContext(nc) as tc:
        with (
            tc.tile_pool(name="const", bufs=1) as constp,
            tc.tile_pool(name="qk", bufs=2) as qkp,
            tc.tile_pool(name="vhp", bufs=2) as vhp,
            tc.tile_pool(name="pt", bufs=16) as ptp,
            tc.tile_pool(name="ep", bufs=2) as epp,
            tc.tile_pool(name="osb", bufs=2) as osbp,
            tc.tile_pool(name="sApsum", bufs=1, space="PSUM") as sAp,
            tc.tile_pool(name="sBpsum", bufs=1, space="PSUM") as sBp,
            tc.tile_pool(name="opsum", bufs=2, space="PSUM") as opsum,
        ):
            wt_sb = constp.tile([D, D], bf16)
            bb_sb = constp.tile([128, NQT * D], f32)
            zb = constp.tile([128, 1], f32)
            nc.vector.memset(zb[:], 0.0)

            def load_pair(pq):
                pa, pb = 2 * pq, 2 * pq + 1
                qt2 = qkp.tile([128, S], bf16, tag="qt", name=f"qt_{pq}")
                kz2 = [
                    qkp.tile([128, S], bf16, tag="kza", name=f"kza_{pq}"),
                    qkp.tile([128, S], bf16, tag="kzb", name=f"kzb_{pq}"),
                ]
                vh2 = vhp.tile([128, 2, NKT, 128], bf16, name=f"vh_{pq}")
                nc.sync.dma_start(qt2[0:D, :], qt_d[pa])
                nc.sync.dma_start(qt2[D:128, :], qt_d[pb])
                nc.sync.dma_start(kz2[0][:], kt_d[pa])
                nc.sync.dma_start(kz2[1][:], kt_d[pb])
                nc.sync.dma_start(vh2[:, 0, :, :], vh_d[pa])
                nc.sync.dma_start(vh2[:, 1, :, :], vh_d[pb])
                return qt2, kz2, vh2

            loaded = load_pair(0)
            # consts are only needed by the (deferred) epilogues; load them
            # after the first pair's inputs so they don't head-of-line block.
            nc.sync.dma_start(wt_sb[:], wt_d[:])
            nc.sync.dma_start(bb_sb[:], bb_d[:])

            # Deferred out-projection of the previous pair: the opj matmuls
            # depend on the epilogue chain (copy->bcast->recip->mult), so
            # emitting them before the next pair's QK stream would stall the
            # PE (and starve ScalarE) at every pair boundary. Instead the
            # PSUM tiles are allocated at pair end (for slot cycling) and the
            # matmuls are drip-fed into the next pair's first kt loop.
            pending = None

            def emit_pending_step(step):
                opj2, on2p, out2p, pap, pbp = pending
                if step <= 4:
                    for x in (0, 1):
                        for t in range(4 * (step - 1), 4 * step):
                            nc.tensor.matmul(
                                opj2[x][:, t * D:(t + 1) * D],
                                on2p[x][:, t * 128:(t + 1) * 128],
                                wt_sb[:],
                                start=True, stop=True,
                            )
                elif step == 5:
                    nc.vector.tensor_tensor(out2p[0][:], opj2[0][:], bb_sb[:], ADD)
                    nc.sync.dma_start(out_d[pap], out2p[0][:])
                elif step == 6:
                    nc.vector.tensor_tensor(out2p[1][:], opj2[1][:], bb_sb[:], ADD)
                    nc.sync.dma_start(out_d[pbp], out2p[1][:])

            for pq in range(PAIRS // 2):
                pa, pb = 2 * pq, 2 * pq + 1
                qt2, kz2, vh2 = loaded
                out2 = [
                    osbp.tile([128, NQT * D], f32, tag="outA", name=f"out_{pq}_A"),
                    osbp.tile([128, NQT * D], f32, tag="outB", name=f"out_{pq}_B"),
                ]
                on2 = [
                    epp.tile([D, S], bf16, tag="onA", name=f"on_{pq}_A"),
                    epp.tile([D, S], bf16, tag="onB", name=f"on_{pq}_B"),
                ]

                for c in range(NCHUNK):
                    q0 = c * CHUNK
                    o_ps = [
                        opsum.tile([128, CHUNK], f32, tag="o", name=f"oA_{pq}_{c}"),
                        opsum.tile([128, CHUNK], f32, tag="o", name=f"oB_{pq}_{c}"),
                    ]
                    for k in range(NKT):
                        s_ps = [
                            sAp.tile([128, CHUNK], f32, tag="s", name=f"sA_{pq}_{c}_{k}"),
                            sBp.tile([128, CHUNK], f32, tag="s", name=f"sB_{pq}_{c}_{k}"),
                        ]
                        # j=0/j=1 share the stationary operand.
                        for x in (0, 1):
                            for j in (0, 1):
                                nc.tensor.matmul(
                                    s_ps[x][:, j * 512:(j + 1) * 512],
                                    kz2[x][:, k * 128:(k + 1) * 128],
                                    qt2[:, q0 + j * 512:q0 + (j + 1) * 512],
                                    start=True, stop=True,
                                )
                        p_sb = [None, None]
                        for x in (0, 1):
                            p_sb[x] = ptp.tile([128, CHUNK], bf16, tag="p", name=f"p_{pq}_{c}_{k}_{x}")
                            nc.scalar.activation(p_sb[x][:], s_ps[x][:], EXPF, bias=zb[:], scale=0.125)
                        for x in (0, 1):
                            for j in (0, 1):
                                nc.tensor.matmul(
                                    o_ps[x][:, j * 512:(j + 1) * 512],
                                    vh2[:, x, k, :],
                                    p_sb[x][:, j * 512:(j + 1) * 512],
                                    start=(k == 0), stop=(k == NKT - 1),
                                )
                        if c == 0 and pending is not None and 1 <= k <= 6:
                            emit_pending_step(k)
                            if k == 6:
                                pending = None

                    if c == NCHUNK - 1 and pq + 1 < PAIRS // 2:
                        loaded = load_pair(pq + 1)

                    for x in (0, 1):
                        o_sb = epp.tile([D + 1, CHUNK], f32, tag="osb", name=f"osb_{pq}_{c}_{x}")
                        nc.vector.tensor_copy(o_sb[:], o_ps[x][0:D + 1, :])
                        rs = epp.tile([1, CHUNK], f32, tag="rs", name=f"rs_{pq}_{c}_{x}")
                        nc.sync.dma_start(rs[:], o_sb[D:D + 1, :])
                        rb = epp.tile([D, CHUNK], f32, tag="rb", name=f"rb_{pq}_{c}_{x}")
                        nc.gpsimd.partition_broadcast(rb[:], rs[:])
                        nc.vector.reciprocal_approx_fast(rb[:], rb[:])
                        nc.vector.tensor_tensor(
                            on2[x][:, q0:q0 + CHUNK], o_sb[0:D, :], rb[:], MULT
                        )
                        if pq == PAIRS // 2 - 1 and c == NCHUNK - 1:
                            opj = opsum.tile(
                                [128, NQT * D], f32, tag="o", name=f"opjL_{x}"
                            )
                            for t in range(NQT):
                                nc.tensor.matmul(
                                    opj[:, t * D:(t + 1) * D],
                                    on2[x][:, t * 128:(t + 1) * 128],
                                    wt_sb[:],
                                    start=True, stop=True,
                                )
                            nc.vector.tensor_tensor(out2[x][:], opj[:], bb_sb[:], ADD)
                            nc.sync.dma_start(out_d[[pa, pb][x]], out2[x][:])

                if pq < PAIRS // 2 - 1:
                    opj2 = [
                        opsum.tile([128, NQT * D], f32, tag="o", name=f"opj_{pq}_A"),
                        opsum.tile([128, NQT * D], f32, tag="o", name=f"opj_{pq}_B"),
                    ]
                    pending = (opj2, on2, out2, pa, pb)

    nc.compile()
    return nc


def kernel(queries, keys, values, W_out, b_out):
    bf16 = ml_dtypes.bfloat16

    q = np.asarray(queries, dtype=np.float32).reshape(B * H, S, D)
    k = np.asarray(keys, dtype=np.float32).reshape(B * H, S, D)
    v = np.asarray(values, dtype=np.float32).reshape(B * H, S, D)

    wt = np.ascontiguousarray(np.asarray(W_out, dtype=np.float32).T).astype(bf16)
    bb = np.ascontiguousarray(
        np.tile(np.asarray(b_out, dtype=np.float32), (128, NQT))
    )

    in_maps = []
    for c in range(NCORES):
        sl = slice(c * PAIRS, (c + 1) * PAIRS)
        qt = np.ascontiguousarray(q[sl].transpose(0, 2, 1)).astype(bf16)
        # K^T zero-padded to 128 contraction rows: even pairs occupy rows
        # 0-63, odd pairs rows 64-127 (matching their slot in the stacked
        # qt2 rhs; the zero rows annihilate the other head's queries).
        kt = np.zeros((PAIRS, 128, S), dtype=bf16)
        for pp in range(PAIRS):
            r0 = (pp % 2) * D
            kt[pp, r0:r0 + D] = k[sl][pp].T.astype(bf16)
        # [pairs, S, D] -> k-tiled p-major [pairs, 128, NKT, 128]: cols 0-63
        # V, col 64 ones (softmax denominator), cols 65-127 zero padding.
        vt = v[sl].reshape(PAIRS, NKT, 128, D).transpose(0, 2, 1, 3)
        vh = np.zeros((PAIRS, 128, NKT, 128), dtype=bf16)
        vh[..., :D] = vt.astype(bf16)
        vh[..., D] = 1.0
        in_maps.append({"qt": qt, "kt": kt, "vh": vh, "wt": wt, "bb": bb})

    if "nc" not in _NC_CACHE:
        _NC_CACHE["nc"] = build_nc()
    nc = _NC_CACHE["nc"]

    global _LAST_IN_MAPS
    _LAST_IN_MAPS = in_maps

    res = run_bass_kernel_spmd(nc, in_maps, list(range(NCORES)))

    out = np.empty((B * H, S, D), dtype=np.float32)
    for c in range(NCORES):
        o = res.results[c]["out"]  # [PAIRS, 128, NQT*D], q = t*128 + p
        out[c * PAIRS:(c + 1) * PAIRS] = (
            o.reshape(PAIRS, 128, NQT, D).transpose(0, 2, 1, 3).reshape(PAIRS, S, D)
        )
    return out.reshape(B, H, S, D)
